# revision 18
# baseline (speedup 1.0000x reference)
"""Trainium2 Bass kernel for nn_BasicDeconvolutionBlock (sparse 3x3x3 transposed
conv + BatchNorm + ReLU) running SPMD on 8 NeuronCores.

Algorithm
---------
The reference computes, for 27 kernel offsets k:
    out[out_map[k]] += feats[in_map[k]] @ w[k]
followed by train-mode BatchNorm over the 250k voxels and ReLU.

Within one offset k the out indices are unique, so the edge lists invert into a
dense per-output gather table:  table[j, k] = input row for output j at offset
k (or a dummy zero row when the neighbor voxel doesn't exist).  Then

    conv[j] = sum_k feats_pad[table[j, k]] @ w[k]

which maps onto the hardware as: dma_gather (transpose mode, bf16, channels on
partitions) -> weight-stationary matmul accumulating all 27 offsets in PSUM.
Because the voxel coords are sorted, table[j,k] stays within +-4160 rows of j,
so per-core gather windows are narrow enough for the gather engine's int16
indices after per-supertile rebasing.

Sharding: 8 cores x 31250 contiguous output rows.  Each core gets a private
copy of the feats rows it needs (shard + halo) in bf16 padded to 128 channels
(gather element size must be a multiple of 256 bytes), with zero rows
interleaved every ZPD rows to serve as dummy gather targets.  BatchNorm mean /
var are computed from per-core partial sums and combined with an AllReduce.
"""

import math
from contextlib import ExitStack

import numpy as np
import ml_dtypes

import concourse.bass as bass
import concourse.bacc as bacc
import concourse.tile as tile
from concourse import mybir

BF16 = ml_dtypes.bfloat16

NCORES = 8
K = 27
CIN = 64
COUT = 64
EPS = 1e-5


class Cfg:
    """Geometry of the sharded gather pipeline.  All sizes in rows/cols."""

    def __init__(self, n, shard, ni, nsup, chunk, halo, zpd, win,
                 single_packet=True):
        self.single_packet = single_packet
        self.N = n              # total voxels
        self.SHARD = shard      # output rows per core (true)
        self.NI = ni            # gather size / supertile columns
        self.NSUP = nsup        # supertiles per core; NSUP*NI >= SHARD
        self.CHUNK = chunk      # matmul N / PSUM chunk columns (<= 512)
        self.HALO = halo        # max |table[j,k] - j| plus margin
        self.ZPD = zpd          # data rows between interleaved zero rows
        self.WIN = win          # gather window rows (int16 index range)
        self.ZP = zpd + 1       # local period incl. the zero row
        self.SHARD_PAD = ni * nsup
        self.NCH = ni // chunk  # chunks per supertile (must be even)
        # local feats length: last window must fit
        self.L = (nsup - 1) * ni + win
        assert ni % 128 == 0 and ni % chunk == 0 and self.NCH % 2 == 0
        assert chunk % 128 == 0 and chunk <= 512
        assert win <= 32768
        assert shard <= self.SHARD_PAD

    def local_of_global(self, g):
        """Index into the per-core local feats array for shard-relative data
        row g (g >= 0 counts data rows from gbase).  Zero rows sit at local
        positions that are multiples of ZP."""
        return g + g // self.ZPD + 1


FULL = Cfg(n=250000, shard=31250, ni=4096, nsup=8, chunk=512,
           halo=4224, zpd=4096, win=16384, single_packet=False)


# ---------------------------------------------------------------------------
# CPU-side preparation
# ---------------------------------------------------------------------------

def build_table(in_map, out_map, n):
    """Dense per-output gather table: table[j,k] = in row or -1."""
    k = in_map.shape[0]
    table = np.full((n + 1, k), -1, dtype=np.int64)
    for kk in range(k):
        table[out_map[kk], kk] = in_map[kk]
    return table[:n]


def prep_core_inputs(cfg, core, feats_bf_pad, table):
    """Per-core local feats array and wrapped int16 gather indices."""
    n = cfg.N
    gbase = core * cfg.SHARD - cfg.HALO  # global row of data index 0

    # local feats: [L, 128] bf16 with zero rows every ZP local rows
    r = np.arange(cfg.L)
    is_zero = (r % cfg.ZP) == 0
    d = r - r // cfg.ZP - 1           # data index for non-zero rows
    g = gbase + d
    valid = (~is_zero) & (g >= 0) & (g < n)
    local = np.zeros((cfg.L, 128), dtype=BF16)
    local[valid] = feats_bf_pad[g[valid]]

    # gather indices per (supertile, offset k)
    j0 = core * cfg.SHARD
    cols = np.arange(cfg.SHARD_PAD)
    jj = j0 + cols
    in_range = jj < min((core + 1) * cfg.SHARD, n)
    tloc = np.full(cfg.SHARD_PAD, -1, dtype=np.int64)

    idx16 = np.empty((cfg.NSUP * K, 128, cfg.NI // 16), dtype=np.int16)
    for s in range(cfg.NSUP):
        base = s * cfg.NI
        sl = slice(base, base + cfg.NI)
        ccols = cols[sl]
        # dummy target: zero row near the expected band position (monotonic).
        # zero rows sit at absolute local positions that are multiples of ZP;
        # indices are window-relative, so subtract the window base.
        zabs = ((cfg.HALO + ccols) // cfg.ZP) * cfg.ZP
        znear = zabs - base
        assert np.all((znear >= 0) & (znear < cfg.WIN))
        for kk in range(K):
            t = np.where(in_range[sl], table[np.minimum(jj[sl], n - 1), kk], -1)
            gg = t - gbase
            lidx = np.where(t >= 0, gg + gg // cfg.ZPD + 1, 0)
            rel = np.where(t >= 0, lidx - base, znear)
            if not (np.all(rel >= 0) and np.all(rel < cfg.WIN)):
                raise AssertionError(
                    f"gather index out of window: core={core} s={s} k={kk} "
                    f"min={rel.min()} max={rel.max()} win={cfg.WIN}")
            wrapped = rel.reshape(cfg.NI // 16, 16).T.astype(np.int16)
            idx16[s * K + kk] = np.tile(wrapped, (8, 1))
    return local, idx16


def make_consts():
    """[128, 192] f32: [:64,:64] identity (transpose rhs), [:,64:] fold2."""
    c = np.zeros((128, 192), dtype=np.float32)
    c[:64, :64] = np.eye(64, dtype=np.float32)
    c[64:, :64] = np.eye(64, dtype=np.float32)
    p = np.arange(128)
    c[:, 64:] = (p[None, :] % 64 == p[:, None] % 64).astype(np.float32)
    return c


def prep_inputs(cfg, feats, w, gamma, beta, in_map, out_map):
    feats = np.asarray(feats, dtype=np.float32)
    w = np.asarray(w, dtype=np.float32)
    gamma = np.asarray(gamma, dtype=np.float32)
    beta = np.asarray(beta, dtype=np.float32)
    in_map = np.asarray(in_map, dtype=np.int64)
    out_map = np.asarray(out_map, dtype=np.int64)

    table = build_table(in_map, out_map, cfg.N)

    feats_bf_pad = np.zeros((cfg.N, 128), dtype=BF16)
    feats_bf_pad[:, :CIN] = feats.astype(BF16)

    w_t = np.ascontiguousarray(w.transpose(1, 0, 2)).astype(BF16)  # [64,27,64]
    consts = make_consts()

    in_maps = []
    for core in range(NCORES):
        local, idx16 = prep_core_inputs(cfg, core, feats_bf_pad, table)
        in_maps.append({
            "feats_local": local,
            "idx16": idx16,
            "w_t": w_t.reshape(CIN, K * COUT),
            "consts": consts,
            "gamma": gamma,
            "beta": beta,
        })
    return in_maps


# ---------------------------------------------------------------------------
# Device kernel (Tile framework)
# ---------------------------------------------------------------------------

def build_kernel(cfg, tc, outs, ins, dbg=False):
    nc = tc.nc
    feats_local = ins["feats_local"]   # [L, 128] bf16 DRAM
    idx16 = ins["idx16"]               # [NSUP*K, 128, NI//16] i16 DRAM
    w_t = ins["w_t"]                   # [64, K*64] bf16 DRAM
    consts = ins["consts"]             # [128, 192] f32 DRAM
    gamma = ins["gamma"]               # [64] f32 DRAM
    beta = ins["beta"]                 # [64] f32 DRAM
    outd = outs["out"]                 # [SHARD_PAD, 64] f32 DRAM

    f32 = mybir.dt.float32
    bf16 = mybir.dt.bfloat16
    NI, CH, NCH, NSUP = cfg.NI, cfg.CHUNK, cfg.NCH, cfg.NSUP
    NPAIR = NCH // 2
    STF = NSUP * NI // 2               # stash free dim
    NT = STF // CH                     # pass-2 tiles
    inv_n = 1.0 / cfg.N

    with ExitStack() as ctx:
        singles = ctx.enter_context(tc.tile_pool(name="singles", bufs=1))
        ipool = ctx.enter_context(tc.tile_pool(name="ipool", bufs=3))
        gpool = ctx.enter_context(tc.tile_pool(name="gpool", bufs=2))
        small = ctx.enter_context(tc.tile_pool(name="small", bufs=1))
        scr = ctx.enter_context(tc.tile_pool(name="scr", bufs=2))
        dram = ctx.enter_context(tc.tile_pool(name="dram", bufs=1, space="DRAM"))

        consts_sb = singles.tile([128, 192], f32)
        nc.sync.dma_start(out=consts_sb[:], in_=consts[:, :])
        w_sb = singles.tile([CIN, K * COUT], bf16)
        nc.sync.dma_start(out=w_sb[:], in_=w_t[:, :])
        ident = consts_sb[:64, 0:64]
        fold2 = consts_sb[:, 64:192]

        stash = singles.tile([128, STF], f32)
        sum_p = small.tile([128, NT], f32)
        sq_p = small.tile([128, NT], f32)

        # ---- pass 1: gather + matmul-accumulate over 27 offsets ----
        with tc.tile_pool(name="psA", bufs=8, space="PSUM") as psA:
            for s in range(NSUP):
                win_ap = feats_local[s * NI: s * NI + cfg.WIN, :]
                # one PSUM bank per chunk; alternate partition halves so the
                # drain copies into the packed stash stay partition-aligned
                pts = [psA.tile([128, CH], f32, tag="pts", name=f"pts{s}_{q}")
                       for q in range(NCH)]
                for kk in range(K):
                    it = ipool.tile([128, NI // 16], mybir.dt.int16, tag="it")
                    nc.sync.dma_start(out=it[:], in_=idx16[s * K + kk, :, :])
                    gb = gpool.tile([128, 1, NI], bf16, tag="gb")
                    nc.gpsimd.dma_gather(
                        gb[:], win_ap, it[:],
                        num_idxs=NI, num_idxs_reg=NI,
                        elem_size=128, transpose=True,
                        single_packet=cfg.single_packet)
                    for ci in range(NCH):
                        half = ci % 2
                        nc.tensor.matmul(
                            out=pts[ci][half * 64:half * 64 + 64, :],
                            lhsT=w_sb[:, kk * COUT:(kk + 1) * COUT],
                            rhs=gb[0:CIN, 0, ci * CH:(ci + 1) * CH],
                            start=(kk == 0), stop=(kk == K - 1),
                            tile_position=(0, half * 64))
                for ci in range(NCH):
                    half = ci % 2
                    f0 = s * (NI // 2) + (ci // 2) * CH
                    nc.vector.tensor_copy(
                        out=stash[half * 64:half * 64 + 64, f0:f0 + CH],
                        in_=pts[ci][half * 64:half * 64 + 64, :])

        if dbg:
            nc.sync.dma_start(out=outs["dbg_stash"][:, :], in_=stash[:])

        # ---- BN statistics ----
        for t in range(NT):
            st = stash[:, t * CH:(t + 1) * CH]
            nc.vector.reduce_sum(out=sum_p[:, t:t + 1], in_=st,
                                 axis=mybir.AxisListType.X)
            sq_scr = scr.tile([128, CH], f32, tag="sq")
            nc.scalar.activation(out=sq_scr[:], in_=st,
                                 func=mybir.ActivationFunctionType.Square,
                                 accum_out=sq_p[:, t:t + 1])
        statsin = small.tile([128, 2], f32)
        nc.vector.reduce_sum(out=statsin[:, 0:1], in_=sum_p[:],
                             axis=mybir.AxisListType.X)
        nc.vector.reduce_sum(out=statsin[:, 1:2], in_=sq_p[:],
                             axis=mybir.AxisListType.X)

        with tc.tile_pool(name="psB", bufs=2, space="PSUM") as psB, \
             tc.tile_pool(name="psC", bufs=4, space="PSUM") as psC, \
             tc.tile_pool(name="p2n", bufs=3) as p2n, \
             tc.tile_pool(name="p2o", bufs=3) as p2o:
            pst = psB.tile([2, 128], f32, tag="pst")
            nc.tensor.matmul(out=pst[:], lhsT=statsin[:], rhs=fold2,
                             start=True, stop=True)
            comb = small.tile([4, 128], f32)
            nc.vector.tensor_copy(out=comb[0:2, :], in_=pst[:])
            nc.sync.dma_start(out=comb[2:3, 0:64], in_=gamma[None, :])
            nc.sync.dma_start(out=comb[2:3, 64:128], in_=gamma[None, :])
            nc.sync.dma_start(out=comb[3:4, 0:64], in_=beta[None, :])
            nc.sync.dma_start(out=comb[3:4, 64:128], in_=beta[None, :])

            # all-reduce the partial sums across the 8 cores
            cc_in = dram.tile([2, 128], f32)
            cc_out = dram.tile([2, 128], f32)
            nc.gpsimd.dma_start(out=cc_in[:], in_=comb[0:2, :])
            nc.gpsimd.collective_compute(
                "AllReduce", mybir.AluOpType.add,
                replica_groups=[list(range(NCORES))],
                ins=[cc_in.opt()], outs=[cc_out.opt()])
            nc.gpsimd.dma_start(out=comb[0:2, :], in_=cc_out[:])

            ptt = psB.tile([128, 4], f32, tag="ptt")
            nc.tensor.transpose(out=ptt[:], in_=comb[:], identity=consts_sb[0:4, 0:4])
            stt = small.tile([128, 4], f32)
            nc.vector.tensor_copy(out=stt[:], in_=ptt[:])
            if dbg:
                nc.sync.dma_start(out=outs["dbg_stt"][:, 0:4], in_=stt[:])
                nc.sync.dma_start(out=outs["dbg_stt"][:, 4:6], in_=statsin[:])

            mean = small.tile([128, 1], f32)
            var = small.tile([128, 1], f32)
            msq = small.tile([128, 1], f32)
            scal = small.tile([128, 1], f32)
            shift = small.tile([128, 1], f32)
            eps_t = small.tile([128, 1], f32)
            nc.vector.memset(eps_t[:], EPS)
            nc.vector.tensor_scalar_mul(out=mean[:], in0=stt[:, 0:1], scalar1=inv_n)
            nc.vector.tensor_scalar_mul(out=var[:], in0=stt[:, 1:2], scalar1=inv_n)
            nc.vector.tensor_tensor(out=msq[:], in0=mean[:], in1=mean[:],
                                    op=mybir.AluOpType.mult)
            nc.vector.tensor_tensor(out=var[:], in0=var[:], in1=msq[:],
                                    op=mybir.AluOpType.subtract)
            nc.scalar.activation(out=var[:], in_=var[:],
                                 func=mybir.ActivationFunctionType.Sqrt,
                                 bias=eps_t[:], scale=1.0)
            nc.vector.reciprocal(out=var[:], in_=var[:])       # rstd
            nc.vector.tensor_tensor(out=scal[:], in0=stt[:, 2:3], in1=var[:],
                                    op=mybir.AluOpType.mult)
            nc.vector.tensor_tensor(out=msq[:], in0=mean[:], in1=scal[:],
                                    op=mybir.AluOpType.mult)
            nc.vector.tensor_tensor(out=shift[:], in0=stt[:, 3:4], in1=msq[:],
                                    op=mybir.AluOpType.subtract)

            # ---- pass 2: normalize + relu + transpose + store ----
            nblk = CH // 128
            for t in range(NT):
                s, q = t // NPAIR, t % NPAIR
                nt_t = p2n.tile([128, CH], f32, tag="nt")
                nc.vector.tensor_scalar(
                    out=nt_t[:], in0=stash[:, t * CH:(t + 1) * CH],
                    scalar1=scal[:], scalar2=shift[:],
                    op0=mybir.AluOpType.mult, op1=mybir.AluOpType.add)
                nc.scalar.activation(out=nt_t[:], in_=nt_t[:],
                                     func=mybir.ActivationFunctionType.Relu)
                for h in range(2):
                    ptr = psC.tile([128, nblk * 64], f32, tag="ptr")
                    for b in range(nblk):
                        nc.tensor.transpose(
                            out=ptr[:, b * 64:(b + 1) * 64],
                            in_=nt_t[h * 64:(h + 1) * 64, b * 128:(b + 1) * 128],
                            identity=consts_sb[h * 64:(h + 1) * 64, 0:64])
                    ob = p2o.tile([128, nblk, 64], f32, tag="ob")
                    nc.vector.tensor_copy(out=ob[:], in_=ptr[:])
                    j0 = s * NI + (2 * q + h) * CH
                    nc.sync.dma_start(
                        out=outd[j0:j0 + CH, :].rearrange("(b p) c -> p b c", p=128),
                        in_=ob[:])


# ---------------------------------------------------------------------------
# Entry point
# ---------------------------------------------------------------------------

def _build_nc(cfg, dbg=False):
    nc = bacc.Bacc("TRN2", target_bir_lowering=False, debug=False,
                   num_devices=NCORES)
    f32 = mybir.dt.float32
    bf16 = mybir.dt.bfloat16
    i16 = mybir.dt.int16
    ins = {
        "feats_local": nc.dram_tensor("feats_local", [cfg.L, 128], bf16,
                                      kind="ExternalInput").ap(),
        "idx16": nc.dram_tensor("idx16", [cfg.NSUP * K, 128, cfg.NI // 16],
                                i16, kind="ExternalInput").ap(),
        "w_t": nc.dram_tensor("w_t", [CIN, K * COUT], bf16,
                              kind="ExternalInput").ap(),
        "consts": nc.dram_tensor("consts", [128, 192], f32,
                                 kind="ExternalInput").ap(),
        "gamma": nc.dram_tensor("gamma", [COUT], f32,
                                kind="ExternalInput").ap(),
        "beta": nc.dram_tensor("beta", [COUT], f32,
                               kind="ExternalInput").ap(),
    }
    outs = {
        "out": nc.dram_tensor("out", [cfg.SHARD_PAD, COUT], f32,
                              kind="ExternalOutput").ap(),
    }
    if dbg:
        outs["dbg_stash"] = nc.dram_tensor(
            "dbg_stash", [128, cfg.NSUP * cfg.NI // 2], f32,
            kind="ExternalOutput").ap()
        outs["dbg_stt"] = nc.dram_tensor(
            "dbg_stt", [128, 6], f32, kind="ExternalOutput").ap()
    with tile.TileContext(nc) as tc:
        build_kernel(cfg, tc, outs, ins, dbg=dbg)
    nc.compile()
    return nc


def _run(cfg, inputs_by_core, trace=False):
    from concourse.bass_utils import run_bass_kernel_spmd

    nc = _build_nc(cfg)
    res = run_bass_kernel_spmd(nc, inputs_by_core, core_ids=list(range(NCORES)),
                               trace=trace)
    return res


def kernel(feats, w, gamma, beta, in_map, out_map, _cfg=FULL, _trace=False,
           _return_res=False):
    cfg = _cfg
    in_maps = prep_inputs(cfg, feats, w, gamma, beta, in_map, out_map)
    res = _run(cfg, in_maps, trace=_trace)
    shards = [res.results[c]["out"][:cfg.SHARD] for c in range(NCORES)]
    out = np.concatenate(shards, axis=0)[:cfg.N].astype(np.float32)
    if _return_res:
        return out, res
    return out
